# revision 37
# baseline (speedup 1.0000x reference)
# Trainium2 Bass kernel: GQA sliding-window attention (JanusSelfAttention).
#
# Problem: B=2, S=2048, D=1024, H=16 q-heads, KH=4 kv-heads, HD=64,
# WINDOW=512 causal band, QK-RMSNorm (weights==1) then RoPE, GQA attention,
# out proj. Full inputs in, full outputs out.
#
# Sharding: 8 shards = (batch, seq quarter of 512 query tokens). Each core
# recomputes the 512-token K/V halo from x (no collectives). The first seq
# chunk's zero-pad halo is killed through the softmax denominator: the
# "ones" row of the V-stationary is a per-(core, kv-tile) constant that is
# zero for pad tiles (V itself is zero there since x is zero), so no exp
# bias is needed and the exp can batch over the whole kv span.
#
# Stage 3 is q-tile-major: per (group, q-tile) the five 128-wide kv-tile
# score matmuls land in two PSUM windows (3 banks + 2 banks) and get TWO
# large exp activations (amortizing the ~300ns ACT instruction overhead),
# then five AV matmuls accumulate into one bank ([ones|V] stationary puts
# the denominator on partition 0). The output projection for q-tile t runs
# as soon as its four groups are normalized, keeping the tensor engine
# dense (HAM stays warm) instead of a separate tail stage.

import numpy as np

B, S, D = 2, 2048, 1024
H, KH, HD = 16, 4, 64
WINDOW = 512
EPS = 1e-5
P = 128
CHUNK = 512          # query tokens per core
TKV = 1024           # kv tokens per core (halo + own)
NCORES = 8
NT = TKV // P        # 8 token tiles (first 4 = halo)
NTQ = CHUNK // P     # 4 own q tiles
ND = D // P          # 8 d chunks
# q-head order in the permuted feature layout: block i holds heads
# (HEAD_ORDER[2i] at partitions 0-63, HEAD_ORDER[2i+1] at 64-127), pairing a
# parity-0 kv-group head with a parity-1 kv-group head.
HEAD_ORDER = [0, 4, 1, 5, 2, 6, 3, 7, 8, 12, 9, 13, 10, 14, 11, 15]
# feature permutation: new feature j comes from old feature QFEAT_PERM[j]
QFEAT_PERM = np.concatenate([np.arange(h * HD, (h + 1) * HD) for h in HEAD_ORDER])
# constants blob column offsets (bf16 cols): rope first (needed in stage 1),
# ident/tri/ones deferred
C_ROPE, C_IDENT, C_TRI0, C_TRI1, C_ONES = 0, 1536, 1664, 2176, 2688
C_TOTAL = 2696
C_MISC = C_IDENT

_built = {}


def _build():
    """Build and compile the SPMD Bass program (same for all 8 cores)."""
    import concourse.bacc as bacc
    import concourse.mybir as mybir
    import concourse.tile as tile

    f32 = mybir.dt.float32
    bf16 = mybir.dt.bfloat16
    AF = mybir.ActivationFunctionType

    nc = bacc.Bacc(
        "TRN2", target_bir_lowering=False, debug=False, enable_asserts=False
    )

    xt2 = nc.dram_tensor("xt2", [P, NT * D], bf16, kind="ExternalInput").ap()
    wqb = nc.dram_tensor("wqb", [P, ND * H * HD], bf16, kind="ExternalInput").ap()
    wkvb = nc.dram_tensor("wkvb", [P, ND * 512], bf16, kind="ExternalInput").ap()
    wob = nc.dram_tensor("wob", [P, ND * D], bf16, kind="ExternalInput").ap()
    cstb = nc.dram_tensor("cstb", [P, C_TOTAL], bf16, kind="ExternalInput").ap()
    out = nc.dram_tensor("out", [CHUNK, D], bf16, kind="ExternalOutput").ap()

    with tile.TileContext(nc, pool_alloc_mode="queue") as tc:
        cst = tc.alloc_tile_pool(name="cst", bufs=1)
        rope_sb = cst.tile([P, 24 * HD], bf16, tag="ropet", name="ropet")
        misc_sb = cst.tile([P, C_TOTAL - C_MISC], bf16, tag="misct", name="misct")
        ident_sb = misc_sb[:, C_IDENT - C_MISC:C_IDENT - C_MISC + P]
        tri_sb = [misc_sb[:, C_TRI0 - C_MISC:C_TRI0 - C_MISC + 512],
                  misc_sb[:, C_TRI1 - C_MISC:C_TRI1 - C_MISC + 512]]
        ones_sb = misc_sb[:, C_ONES - C_MISC:C_ONES - C_MISC + NT]
        cq_sb = rope_sb[:, 0:4 * HD]
        sq_sb = rope_sb[:, 4 * HD:8 * HD]
        ck_sb = rope_sb[:, 8 * HD:16 * HD]
        sk_sb = rope_sb[:, 16 * HD:24 * HD]
        epsq_sb = cst.tile([P, 1], f32, tag="epsq", name="epsq")
        nc.vector.memset(epsq_sb[:], float(HD * EPS))
        epsk_sb = cst.tile([P, 1], f32, tag="epsk", name="epsk")
        nc.vector.memset(epsk_sb[:], float(EPS))

        # ---- pools ordered by lifetime ----
        wow = tc.alloc_tile_pool(name="wow", bufs=1)
        s2a = tc.alloc_tile_pool(name="s2a", bufs=1)
        # at_all: [128, (fb, tq, 128)] bf16 - wo stationary slices
        at_all = s2a.tile([P, ND * NTQ * P], bf16, tag="at", name="at")
        s2 = tc.alloc_tile_pool(name="s2qk", bufs=1)
        # Q^T group-interleaved: qtg[gg] rows 0-63 = group 2gg, 64-127 =
        # group 2gg+1; cols = (qtile, head4, 128)
        qtg = [s2.tile([P, NTQ * 4 * P], bf16, tag=f"qtg{i}", name=f"qtg{i}")
               for i in range(2)]
        # K^T: [128, ((j,b), 128)]; rows 0-63 groups 0/2, 64-127 groups 1/3
        kt_all = s2.tile([P, 2 * NT * P], bf16, tag="kt", name="kt")
        sv = tc.alloc_tile_pool(name="sv", bufs=1)
        s1 = tc.alloc_tile_pool(name="s1o", bufs=1)
        s1w = tc.alloc_tile_pool(name="s1w", bufs=1)
        xcp = tc.alloc_tile_pool(name="xcp", bufs=8)
        wq_sb = s1w.tile([P, ND * H * HD], bf16, tag="wq", name="wq")
        wo_sb = wow.tile([P, ND * D], bf16, tag="wo", name="wo")
        # Head DMA priority: the sync FIFO carries the critical bytes in
        # exactly the order stage 1 consumes them (x0 halves + wkv halves,
        # then x1..x7); wq waits for x1/x2 to land so its 2MB doesn't steal
        # HBM bandwidth from the early tiles.
        wkv_sb = [s1w.tile([P, 2048], bf16, tag=f"wkv{h}", name=f"wkv{h}")
                  for h in range(2)]
        xcols = [xcp.tile([P, D], bf16, tag="xp", name="xp") for t in range(NT)]
        nc.sync.dma_start(xcols[0][:, 0:512], xt2[:, 0:512])
        nc.sync.dma_start(wkv_sb[0][:], wkvb[:, 0:2048])
        nc.sync.dma_start(xcols[0][:, 512:D], xt2[:, 512:D])
        nc.sync.dma_start(wkv_sb[1][:], wkvb[:, 2048:4096])
        for t in range(1, NT):
            nc.sync.dma_start(xcols[t][:], xt2[:, t * D:(t + 1) * D])
        # rope constants early on gpsimd (needed by tile-0's k-rope)
        nc.gpsimd.dma_start(rope_sb[:], cstb[:, C_ROPE:C_ROPE + 24 * HD])
        nc.gpsimd.dma_start(misc_sb[:], cstb[:, C_MISC:])
        # wq gated behind x0/x1 landings (needed from tile 4 on)
        nc.vector.tensor_copy(wq_sb[0:1, 0:1], xcols[0][0:1, 512:513])
        nc.vector.tensor_copy(wq_sb[0:1, ND * 512:ND * 512 + 1],
                              xcols[1][0:1, 0:1])
        nc.scalar.dma_start(wq_sb[:, 0:ND * 512], wqb[:, 0:ND * 512])
        nc.scalar.dma_start(wq_sb[:, ND * 512:], wqb[:, ND * 512:])

        q_sb = [s1.tile([P, H * HD], bf16, tag=f"q{t}", name=f"q{t}")
                for t in range(NTQ)]
        k_sb = [s1.tile([P, KH * HD], bf16, tag=f"k{t}", name=f"k{t}")
                for t in range(NT)]
        # V stationary per head: [ones | 63 zeros | V(64)] so the AV matmul
        # puts the softmax denominator on PSUM partition 0 (readable by
        # reciprocal_approx_fast directly) and values on partitions 64-127.
        # The "ones" value is a per-(core, tile) constant: zero on pad tiles.
        v_sb = [sv.tile([P, KH * P], bf16, tag=f"v{t}", name=f"v{t}")
                for t in range(NT)]

        # pk first so the stage-3 score window (5 banks) lands mostly on pk
        # banks, which free early (tile-7's K chain runs before its Q chain)
        pk = tc.alloc_tile_pool(name="pk", bufs=4, space="PSUM")
        pq = tc.alloc_tile_pool(name="pq", bufs=2, space="PSUM")
        tp = tc.alloc_tile_pool(name="tp", bufs=2, space="PSUM")
        tmp = tc.alloc_tile_pool(name="tmp", bufs=2)
        sst = tc.alloc_tile_pool(name="sst", bufs=4)

        def rope(eng12, eng34, dst_ap, cos_ap, sin_ap, nh):
            # dst [P, nh*HD] in-place; cos/sin [P, HD] (pair-expanded,
            # sign-folded); muls 1-2 on eng12, mul3+add on eng34
            t2 = tmp.tile([P, nh * HD], bf16, tag=f"rope{nh}", name=f"rope{nh}")
            qa = dst_ap.rearrange("p (h d) -> p h d", h=nh)
            qb = dst_ap.rearrange("p (h w two) -> p h w two", h=nh, two=2)
            t2b = t2[:].rearrange("p (h w two) -> p h w two", h=nh, two=2)
            cosb = cos_ap.unsqueeze(1).broadcast_to([P, nh, HD])
            sin2 = sin_ap.rearrange("p (w two) -> p w two", two=2)
            sin_e = sin2[:, :, 0].unsqueeze(1).broadcast_to([P, nh, HD // 2])
            sin_o = sin2[:, :, 1].unsqueeze(1).broadcast_to([P, nh, HD // 2])
            eng12.tensor_mul(t2b[:, :, :, 0], qb[:, :, :, 1], sin_e)
            eng12.tensor_mul(t2b[:, :, :, 1], qb[:, :, :, 0], sin_o)
            eng34.tensor_mul(qa, qa, cosb)
            eng34.tensor_add(dst_ap, dst_ap, t2[:])

        # k t-pairs (2i,2i+1) -> kt_all[:, i*512:+512]
        def ktrans(i, pool, tag, ceng):
            tpp = pool.tile([P, 512], bf16, tag=tag, name="tpp")
            for half in range(2):
                for bb in range(2):
                    nc.tensor.transpose(
                        tpp[:, (half * 2 + bb) * P:(half * 2 + bb + 1) * P],
                        k_sb[2 * i + half][:, bb * P:(bb + 1) * P], ident_sb)
            if ceng is nc.scalar:
                ceng.copy(kt_all[:, i * 512:(i + 1) * 512], tpp[:])
            else:
                ceng.tensor_copy(kt_all[:, i * 512:(i + 1) * 512], tpp[:])

        def qtrans(tq, pool, tag):
            for gg in range(2):
                tpp = pool.tile([P, 512], bf16, tag=tag, name="tpp")
                for i in range(4):
                    fb = gg * 4 + i
                    nc.tensor.transpose(
                        tpp[:, i * P:(i + 1) * P],
                        q_sb[tq][:, fb * P:(fb + 1) * P], ident_sb)
                nc.vector.tensor_copy(qtg[gg][:, tq * 512:(tq + 1) * 512], tpp[:])

        # ---- PE warmup: ~12 dummy matmuls on scratch data keep the PE busy
        # while the first DMAs land, so the HAM clock gate opens (1.2 ->
        # 2.4 GHz) before the first real projection matmul ----
        wup_sb = cst.tile([P, 512], bf16, tag="wup", name="wup")
        nc.vector.memset(wup_sb[:], 0.0)
        wup_ps = pq.tile([P, 512], f32, tag="pq", name="wup_ps")
        for _ in range(12):
            nc.tensor.matmul(wup_ps[:], wup_sb[:, 0:P], wup_sb[:],
                             start=True, stop=True)

        # ---- stage 1: projections + norm + rope (+ k transposes inline) ----
        for t in range(NT):
            own = t >= NT - NTQ
            tq = t - (NT - NTQ)
            xcol = xcols[t]
            # kv group first (only needs wkv, which lands earliest), then the
            # two q halves as sequential accumulation groups (pq: 2 banks)
            ps = [pk.tile([P, 512], f32, tag="pkv", name="pkv")]
            rhss = [[wkv_sb[d // 4][:, (d % 4) * 512:(d % 4 + 1) * 512]
                     for d in range(ND)]]
            if own:
                ps.append(pq.tile([P, 512], f32, tag="pq", name="pq0"))
                rhss.append([wq_sb[:, d * 1024:d * 1024 + 512] for d in range(ND)])
                ps.append(pq.tile([P, 512], f32, tag="pq", name="pq1"))
                rhss.append([wq_sb[:, d * 1024 + 512:(d + 1) * 1024]
                             for d in range(ND)])
            for pi, pt_ in enumerate(ps):
                for d in range(ND):
                    lhsT = xcol[:, d * P:(d + 1) * P]
                    nc.tensor.matmul(pt_[:], lhsT, rhss[pi][d],
                                     start=(d == 0), stop=(d == ND - 1))
            ps = ps[1:] + ps[:1]   # norm code below expects [q0, q1, kv]
            # k transposes trail the ropes by a few tiles so the in-order PE
            # stream never stalls on a pending gpsimd rope; they also fill
            # the pq-recycle bubble at own-tile boundaries
            if t == 4:
                ktrans(0, tp, "tp", nc.scalar)
            elif t == 5:
                ktrans(1, tp, "tp", nc.scalar)
            elif t == 7:
                ktrans(2, tp, "tp", nc.scalar)

            def qnorm_block():
                # Q RMSNorm: inv = 1/sqrt(sumsq + 64*eps) == 0.125/sqrt(mean+eps)
                ss = sst.tile([P, H], f32, tag="ssq", name="ssq")
                inv = sst.tile([P, H], f32, tag="invq", name="invq")
                for b in range(2):
                    sq = tmp.tile([P, 512], f32, tag="sq", name="sq")
                    nc.scalar.activation(sq[:], ps[b][:], AF.Square)
                    nc.vector.reduce_sum(
                        out=ss[:, 8 * b:8 * b + 8].unsqueeze(2),
                        in_=sq[:].rearrange("p (h d) -> p h d", h=8),
                        axis=mybir.AxisListType.X)
                nc.scalar.activation(inv[:], ss[:], AF.Sqrt, bias=epsq_sb[:])
                nc.vector.reciprocal_approx_fast(inv[:], inv[:])
                for b in range(2):
                    nc.vector.tensor_mul(
                        q_sb[tq][:, 512 * b:512 * (b + 1)].rearrange(
                            "p (h d) -> p h d", h=8),
                        ps[b][:].rearrange("p (h d) -> p h d", h=8),
                        inv[:, 8 * b:8 * b + 8].unsqueeze(2).broadcast_to([P, 8, HD]))
                rope(nc.gpsimd, nc.vector, q_sb[tq][:],
                     cq_sb[:, tq * HD:(tq + 1) * HD],
                     sq_sb[:, tq * HD:(tq + 1) * HD], H)
                return inv

            def knorm_block():
                # K RMSNorm: inv = 1/sqrt(sumsq/64 + eps)
                pkv = ps[-1]
                ssk = sst.tile([P, KH], f32, tag="ssk", name="ssk")
                invk = sst.tile([P, KH], f32, tag="invk", name="invk")
                sqk = tmp.tile([P, KH * HD], f32, tag="sqk", name="sqk")
                nc.scalar.activation(sqk[:], pkv[:, 0:KH * HD], AF.Square)
                nc.vector.reduce_sum(out=ssk[:].unsqueeze(2),
                                     in_=sqk[:].rearrange("p (h d) -> p h d", h=KH),
                                     axis=mybir.AxisListType.X)
                nc.scalar.activation(invk[:], ssk[:], AF.Sqrt, scale=1.0 / HD,
                                     bias=epsk_sb[:])
                nc.vector.reciprocal_approx_fast(invk[:], invk[:])
                nc.vector.tensor_mul(
                    k_sb[t][:].rearrange("p (h d) -> p h d", h=KH),
                    pkv[:, 0:KH * HD].rearrange("p (h d) -> p h d", h=KH),
                    invk[:].unsqueeze(2).broadcast_to([P, KH, HD]))
                rope(nc.gpsimd, nc.gpsimd, k_sb[t][:],
                     ck_sb[:, t * HD:(t + 1) * HD],
                     sk_sb[:, t * HD:(t + 1) * HD], KH)
                va = v_sb[t][:].rearrange("p (h e) -> p h e", h=KH)
                nc.vector.memset(va[:, :, 0:HD], 0.0)
                nc.vector.tensor_copy(
                    va[:, :, 0:1],
                    ones_sb[:, t:t + 1].unsqueeze(1).broadcast_to([P, KH, 1]))
                nc.scalar.copy(
                    va[:, :, HD:2 * HD],
                    pkv[:, KH * HD:2 * KH * HD].rearrange("p (h d) -> p h d", h=KH))

            if t == 7:
                # last tile runs K first: its pkv readers drain while the Q
                # matmuls still stream, so the pk banks (score window home)
                # free right at the stage-3 handoff
                knorm_block()
                inv7 = qnorm_block()
                # last table-set user of stage 1 (pinned after the final
                # Sqrt via the data dep on inv7); pulls the exp table load
                # off the first real exp's critical path
                nc.scalar.activation(wup_sb[0:1, 0:1], inv7[0:1, 0:1], AF.Exp)
            else:
                if own:
                    qnorm_block()
                knorm_block()
            if t == 4:
                # gate the 2MB wo fetch behind tile-4's k so it stays out of
                # the critical early DMA window (WAW dep via the 1-elem copy)
                nc.gpsimd.tensor_copy(wo_sb[0:1, 0:1], k_sb[4][0:1, 0:1])
                nc.gpsimd.dma_start(wo_sb[:], wob)

        # ---- stage 2 remainder: transposes that fill the PE while the
        # stage-1 PSUM pools drain their last norm/v-copy readers ----
        qtrans(0, tp, "tp")
        qtrans(1, tp, "tp")
        ktrans(3, tp, "tp", nc.scalar)

        # ---- stage 3+4: attention, q-tile-major, with inline out-proj ----
        sst.release()
        tmp.release()
        xcp.release()
        s1w.release()
        s1.release()
        tp.release()
        pq.release()
        pk.release()

        # PSUM: scwA 3 + scwB 2 + av 2 + wop 1 = 8 banks
        scp = tc.alloc_tile_pool(name="scp", bufs=1, space="PSUM")
        avp = tc.alloc_tile_pool(name="avp", bufs=2, space="PSUM")
        wop = tc.alloc_tile_pool(name="wop", bufs=1, space="PSUM")
        ptp = tc.alloc_tile_pool(name="ptp", bufs=2)
        rcp = tc.alloc_tile_pool(name="rcp", bufs=3)
        osb = tc.alloc_tile_pool(name="osb", bufs=2)

        at4 = at_all[:].rearrange("p (f tq c) -> p f tq c", tq=NTQ, c=P)

        ot_tiles = {}

        def outproj_half(t, c, last=False):
            # half an out projection for q-tile t; the two halves are spread
            # over two pipeline steps so the PE load stays under the exp rate.
            # the drain borrows an avp bank so its halves run back-to-back
            if c == 0:
                ot_tiles[t] = osb.tile([P, D], bf16, tag="ot", name="ot")
            ot = ot_tiles[t]
            if last and c == 1:
                wp = avp.tile([P, 512], f32, tag="av", name="wp2")
            else:
                wp = wop.tile([P, 512], f32, tag="wp", name="wp")
            for f in range(ND):
                nc.tensor.matmul(
                    wp[:],
                    at_all[:, (f * NTQ + t) * P:(f * NTQ + t + 1) * P],
                    wo_sb[:, f * D + c * 512:f * D + (c + 1) * 512],
                    start=(f == 0), stop=(f == ND - 1))
            nc.vector.tensor_copy(ot[:, c * 512:(c + 1) * 512], wp[:])
            if c == 1:
                oeng = [nc.sync, nc.gpsimd, nc.scalar, nc.sync][t]
                oeng.dma_start(out[t * P:(t + 1) * P, :], ot[:])

        def emit_av(st):
            # AV blocks [3,0,4] + normalize for a pending (t, g) step
            t, g, avt, ptB, ptm0, ptm4 = st
            gg, rok = g // 2, (g % 2) * 64
            vj = [v_sb[t + ji][:].rearrange("p (h e) -> p h e", h=KH)[:, g, :]
                  for ji in range(5)]
            nc.tensor.matmul(avt[:], vj[3], ptB[:, 0:512], start=False, stop=False)
            nc.tensor.matmul(avt[:], vj[0], ptm0[:], start=False, stop=False)
            nc.tensor.matmul(avt[:], vj[4], ptm4[:], start=False, stop=True)
            # normalize: 1/denominator, broadcast, scale into at_all
            rc = rcp.tile([1, 512], f32, tag="rc", name="rc")
            nc.vector.reciprocal_approx_fast(rc[:], avt[0:1, :])
            rcb = rcp.tile([HD, 512], f32, tag="rcb", name="rcb")
            nc.gpsimd.partition_broadcast(rcb[:], rc[:])
            nc.vector.tensor_mul(
                at4[rok:rok + 64, gg * 4:(gg + 1) * 4, t, :],
                avt[HD:2 * HD, :].rearrange("p (h c) -> p h c", c=P),
                rcb[:].rearrange("p (h c) -> p h c", c=P))

        # software-pipelined (t, g) sequence: scores for step i interleave
        # with AV matmuls of step i-1 so the exp stream on ACT never gaps
        pend = None
        for t in range(NTQ):
            for g in range(4):
                gg, rok = g // 2, (g % 2) * 64
                vj = [v_sb[t + ji][:].rearrange("p (h e) -> p h e", h=KH)[:, g, :]
                      for ji in range(5)]
                # five kv tiles j = t..t+4 split into a 3-bank + 2-bank window
                scwA = scp.tile([P, 3 * 512], f32, tag="scwA", name="scwA")
                scwB = scp.tile([P, 2 * 512], f32, tag="scwB", name="scwB")
                qmv = qtg[gg][rok:rok + 64, t * 512:(t + 1) * 512]
                for ji in range(3):
                    j = t + ji
                    nc.tensor.matmul(
                        scwA[:, ji * 512:(ji + 1) * 512],
                        kt_all[rok:rok + 64, (2 * j + gg) * P:(2 * j + gg + 1) * P],
                        qmv, start=True, stop=True)
                if g == 1 and t > 0:
                    outproj_half(t - 1, 0)
                elif g == 2 and t > 0:
                    outproj_half(t - 1, 1)
                # remaining q transposes slot into the ACT-bound phase (the
                # wop bank is free between out-projections)
                if (t, g) == (0, 3):
                    qtrans(2, wop, "wp")
                elif (t, g) == (1, 3):
                    qtrans(3, wop, "wp")
                if pend is not None:
                    # middle AV blocks of the previous step (need only its expA)
                    pt_, pg, pavt = pend[6], pend[1], pend[2]
                    pvj = [v_sb[pend[0] + ji][:].rearrange(
                        "p (h e) -> p h e", h=KH)[:, pg, :] for ji in range(5)]
                    nc.tensor.matmul(pavt[:], pvj[1], pt_[:, 512:1024],
                                     start=True, stop=False)
                    nc.tensor.matmul(pavt[:], pvj[2], pt_[:, 1024:1536],
                                     start=False, stop=False)
                for ji in range(3, 5):
                    j = t + ji
                    nc.tensor.matmul(
                        scwB[:, (ji - 3) * 512:(ji - 2) * 512],
                        kt_all[rok:rok + 64, (2 * j + gg) * P:(2 * j + gg + 1) * P],
                        qmv, start=True, stop=True)
                if pend is not None:
                    emit_av((pend[0], pend[1], pend[2], pend[3], pend[4], pend[5]))
                ptA = ptp.tile([P, 3 * 512], bf16, tag="ptA", name="ptA")
                ptB = ptp.tile([P, 2 * 512], bf16, tag="ptB", name="ptB")
                nc.scalar.activation(ptA[:], scwA[:], AF.Exp)
                nc.scalar.activation(ptB[:], scwB[:], AF.Exp)
                # band triangles on the two boundary tiles (ji=0 anti, ji=4
                # causal); separate tiles so middle AV matmuls start sooner
                ptm0 = ptp.tile([P, 512], bf16, tag="ptm0", name="ptm0")
                ptm4 = ptp.tile([P, 512], bf16, tag="ptm4", name="ptm4")
                nc.vector.tensor_mul(ptm0[:], ptA[:, 0:512], tri_sb[0])
                nc.vector.tensor_mul(ptm4[:], ptB[:, 512:1024], tri_sb[1])
                avt = avp.tile([P, 512], f32, tag="av", name="av")
                pend = (t, g, avt, ptB, ptm0, ptm4, ptA)
        # drain the last step
        t, g, avt, ptB, ptm0, ptm4, ptA = pend
        vj = [v_sb[t + ji][:].rearrange("p (h e) -> p h e", h=KH)[:, g, :]
              for ji in range(5)]
        nc.tensor.matmul(avt[:], vj[1], ptA[:, 512:1024], start=True, stop=False)
        nc.tensor.matmul(avt[:], vj[2], ptA[:, 1024:1536], start=False, stop=False)
        emit_av((t, g, avt, ptB, ptm0, ptm4))
        outproj_half(NTQ - 1, 0, last=True)
        outproj_half(NTQ - 1, 1, last=True)

        osb.release()
        rcp.release()
        ptp.release()
        wop.release()
        avp.release()
        scp.release()
        sv.release()
        s2.release()
        s2a.release()
        wow.release()
        cst.release()

    nc.compile()
    return nc


def _host_inputs(x, freqs_cos, freqs_sin, wq, wk, wv, wo):
    """Build the 8 per-core input maps (host-side prep: transpose/pad/expand)."""
    import ml_dtypes

    bf = ml_dtypes.bfloat16
    x = np.asarray(x, np.float32)
    freqs_cos = np.asarray(freqs_cos, np.float32)
    freqs_sin = np.asarray(freqs_sin, np.float32)
    wqT = np.asarray(wq, np.float32).T[:, QFEAT_PERM]
    wkvT = np.concatenate([np.asarray(wk, np.float32).T,
                           np.asarray(wv, np.float32).T], axis=1)
    woT = np.asarray(wo, np.float32).T[QFEAT_PERM, :]
    wqb = np.ascontiguousarray(
        wqT.reshape(ND, P, H * HD).transpose(1, 0, 2).reshape(P, ND * H * HD)
    ).astype(bf)
    wkvb = np.ascontiguousarray(
        wkvT.reshape(ND, P, 512).transpose(1, 0, 2).reshape(P, ND * 512)
    ).astype(bf)
    wob = np.ascontiguousarray(
        woT.reshape(ND, P, D).transpose(1, 0, 2).reshape(P, ND * D)
    ).astype(bf)

    # constants blob: [rope | ident | tri0(anti) | tri1(caus) | ones]
    ki = np.arange(P)[:, None]
    qi = np.arange(P)[None, :]
    anti = (ki > qi).astype(np.float32)
    caus = (ki <= qi).astype(np.float32)
    tri = np.zeros((2, P, 512), np.float32)
    for h4 in range(4):
        tri[0, :, h4 * P:(h4 + 1) * P] = anti
        tri[1, :, h4 * P:(h4 + 1) * P] = caus

    def rope_tabs(pos):
        # pos: [T] global positions (may be <0 for pad; rows zeroed)
        T = len(pos)
        c2 = np.zeros((T, HD), np.float32)
        s2 = np.zeros((T, HD), np.float32)
        val = pos >= 0
        pv = pos[val]
        c = freqs_cos[pv]            # [n, 32]
        s = freqs_sin[pv]
        c2[val, 0::2] = c
        c2[val, 1::2] = c
        s2[val, 0::2] = -s
        s2[val, 1::2] = s
        return c2, s2

    in_maps = []
    for core in range(NCORES):
        b, ch = core // 4, core % 4
        q0 = ch * CHUNK
        k0 = q0 - WINDOW
        # x: [p, (t, c, tok)]: xt2[p, t*1024 + c*128 + tok] = x[b, tok_g, c*128+p]
        xh = np.zeros((TKV, D), np.float32)
        lo = max(0, k0)
        xh[lo - k0:] = x[b, lo:k0 + TKV]
        xt2 = np.ascontiguousarray(
            xh.reshape(NT, P, ND, P).transpose(3, 0, 2, 1).reshape(P, NT * D)
        ).astype(bf)
        kpos = np.arange(k0, k0 + TKV)
        qpos = np.arange(q0, q0 + CHUNK)
        ck2, sk2 = rope_tabs(kpos)
        cq2, sq2 = rope_tabs(qpos)
        ropeblob = np.concatenate([
            cq2.reshape(NTQ, P, HD).transpose(1, 0, 2).reshape(P, NTQ * HD),
            sq2.reshape(NTQ, P, HD).transpose(1, 0, 2).reshape(P, NTQ * HD),
            ck2.reshape(NT, P, HD).transpose(1, 0, 2).reshape(P, NT * HD),
            sk2.reshape(NT, P, HD).transpose(1, 0, 2).reshape(P, NT * HD),
        ], axis=1)
        onesval = np.ones((P, NT), np.float32)
        if ch == 0:
            onesval[:, 0:4] = 0.0
        cstb = np.zeros((P, C_TOTAL), bf)
        cstb[:, C_IDENT:C_IDENT + P] = np.eye(P, dtype=bf)
        cstb[:, C_TRI0:C_TRI0 + 512] = tri[0].astype(bf)
        cstb[:, C_TRI1:C_TRI1 + 512] = tri[1].astype(bf)
        cstb[:, C_ROPE:C_ROPE + 24 * HD] = ropeblob.astype(bf)
        cstb[:, C_ONES:C_ONES + NT] = onesval.astype(bf)
        in_maps.append({
            "xt2": xt2, "wqb": wqb, "wkvb": wkvb, "wob": wob,
            "cstb": np.ascontiguousarray(cstb),
        })
    return in_maps


def kernel(x, freqs_cos, freqs_sin, wq, wk, wv, wo, q_norm_w, k_norm_w):
    from concourse.bass_utils import run_bass_kernel_spmd

    if "nc" not in _built:
        _built["nc"] = _build()
    nc = _built["nc"]
    in_maps = _host_inputs(x, freqs_cos, freqs_sin, wq, wk, wv, wo)
    res = run_bass_kernel_spmd(nc, in_maps, core_ids=list(range(NCORES)))
    y = np.zeros((B, S, D), np.float32)
    for core in range(NCORES):
        b, ch = core // 4, core % 4
        y[b, ch * CHUNK:(ch + 1) * CHUNK] = np.asarray(
            res.results[core]["out"], dtype=np.float32)
    return y
